# revision 14
# baseline (speedup 1.0000x reference)
"""Trainium2 Bass kernel for nn_Attn_69776038691596.

reference computes:
    proj     = einsum('bsh,kh->bsk', enc, W) + bias          # (B,S,H)
    energies = einsum('bh,bsh->bs', hid, proj)               # (B,S)
    out      = softmax(energies, axis=0)                     # over batch

Algebraic rewrite (exact in real arithmetic):
    u[b,:] = hid[b,:] @ W          # (B,H)  -- tiny matmul
    c[b]   = hid[b,:] . bias       # (B,)
    energies[b,s] = enc[b,s,:] . u[b,:] + c[b]

This turns a 275-GFLOP matmul into a 0.27-GFLOP weighted reduction that is
bound by reading encoder_output (512 MB) from HBM once.

Sharding: split the S axis (2048 -> 8 x 256) across the 8 cores. The softmax
runs over the batch axis, which every core holds entirely, so no collectives
are needed.

Per-core layout: partition p = ds*64 + b packs (batch, s-parity); each SBUF
tile (128, H) holds enc[b, 2j+ds, :] rows for one j. A single fused DVE
tensor_tensor_reduce per tile computes E[p, j] = sum_h enc*u2 + c2 (the
per-partition scalar init folds in c). Softmax then runs on the tiny
(128,128) energy tile via PE transpose.
"""
import sys

sys.path.insert(0, "/opt/trn_rl_repo")

import numpy as np

B, S, H = 64, 2048, 1024
N_CORES = 8
S_LOC = S // N_CORES  # 256

_CACHE = {}


def build_nc(s_loc=S_LOC):
    """Build + compile the per-core Bass module. s_loc must be divisible by 4."""
    import concourse.bass as bass
    import concourse.bacc as bacc
    import concourse.tile as tile
    from concourse import mybir
    from concourse.masks import make_identity
    from contextlib import ExitStack

    f32 = mybir.dt.float32
    Alu = mybir.AluOpType
    Act = mybir.ActivationFunctionType
    X = mybir.AxisListType.X

    nj = s_loc // 2          # s-pairs per core (E columns)

    nc = bacc.Bacc("TRN2", target_bir_lowering=False, debug=False,
                   num_devices=N_CORES)
    enc = nc.dram_tensor("enc", [B, s_loc, H], f32, kind="ExternalInput").ap()
    hid = nc.dram_tensor("hid", [B, H], f32, kind="ExternalInput").ap()
    W = nc.dram_tensor("W", [H, H], f32, kind="ExternalInput").ap()
    bias = nc.dram_tensor("bias", [1, H], f32, kind="ExternalInput").ap()
    out = nc.dram_tensor("out", [B, s_loc], f32, kind="ExternalOutput").ap()

    with ExitStack() as ctx:
        tc = ctx.enter_context(tile.TileContext(nc))
        singles = ctx.enter_context(tc.tile_pool(name="singles", bufs=1))
        wpool = ctx.enter_context(tc.tile_pool(name="wpool", bufs=2))
        chunks = ctx.enter_context(tc.tile_pool(name="chunks", bufs=6))
        small = ctx.enter_context(tc.tile_pool(name="small", bufs=1))
        psum = ctx.enter_context(tc.tile_pool(name="psum", bufs=2, space="PSUM"))
        psum1 = ctx.enter_context(tc.tile_pool(name="psum1", bufs=1, space="PSUM"))

        # ---------- phase 0: u2 (128,H), c2 (128,1) ----------
        ident64 = singles.tile([64, 64], f32, tag="ident64")
        make_identity(nc, ident64)
        ident128 = singles.tile([128, 128], f32, tag="ident128")
        make_identity(nc, ident128)

        hid_sb = singles.tile([64, H], f32, tag="hid_sb")
        nc.sync.dma_start(out=hid_sb, in_=hid)

        # bias broadcast to 128 partitions (partition-stride-0 DMA)
        bias_b = singles.tile([128, H], f32, tag="bias_b")
        bias_bc = bass.AP(tensor=bias.tensor, offset=bias.offset,
                          ap=[[0, 128]] + list(bias.ap[1:]))
        nc.sync.dma_start(out=bias_b, in_=bias_bc)

        # hid duplicated on partitions 0-63 / 64-127
        hid2 = singles.tile([128, H], f32, tag="hid2")
        nc.sync.dma_start(out=hid2[0:64, :], in_=hid)
        nc.sync.dma_start(out=hid2[64:128, :], in_=hid)

        # c2[p] = hid2[p,:] . bias ; c2d = c2 / H (folded into the
        # per-element activation bias of the reduction below)
        c2 = singles.tile([128, 1], f32, tag="c2")
        scr = singles.tile([128, H], f32, tag="scr")
        nc.vector.tensor_mul(scr, hid2, bias_b)
        nc.vector.tensor_reduce(c2, scr, axis=X, op=Alu.add)
        c2d = singles.tile([128, 1], f32, tag="c2d")
        nc.vector.tensor_scalar_mul(c2d, c2, 1.0 / H)

        # hidT2[k] = [hid[:,k*128:(k+1)*128].T | same] : (128k, 128p)
        hidT2 = []
        for k in range(8):
            pt = psum.tile([128, 64], f32, tag="pt_hidT")
            nc.tensor.transpose(pt, hid_sb[:, k * 128:(k + 1) * 128], ident64)
            st = singles.tile([128, 128], f32, tag=f"hidT2_{k}")
            nc.vector.tensor_copy(st[:, 0:64], pt)
            nc.vector.tensor_copy(st[:, 64:128], pt)
            hidT2.append(st)

        # u2 = [hid@W ; hid@W] : (128, H) via PE, accumulated over k in PSUM
        u2p = psum1.tile([128, H], f32, tag="u2p")
        for k in range(8):
            wk = wpool.tile([128, H], f32, tag="wk")
            nc.sync.dma_start(out=wk, in_=W[k * 128:(k + 1) * 128, :])
            for nh in range(2):
                nc.tensor.matmul(
                    u2p[:, nh * 512:(nh + 1) * 512],
                    lhsT=hidT2[k],
                    rhs=wk[:, nh * 512:(nh + 1) * 512],
                    start=(k == 0), stop=(k == 7))
        # u2x = u2 duplicated along a 2-wide free dim (matches 2-s-row tiles)
        u2x = singles.tile([128, 2, H], f32, tag="u2x")
        nc.vector.tensor_copy(u2x[:, 0, :], u2p)
        nc.vector.tensor_copy(u2x[:, 1, :], u2p)

        # ---------- phase 1: energies E[p, j] ----------
        E = singles.tile([128, nj], f32, tag="E")
        # Tile t holds s = 4t + 2*ds + r : partition p = ds*64 + b, free (r, h).
        # Per tile, two DMAs (one per ds half / partition half) so descriptors
        # fan out across the SDMA engines serving that half (a leading dim of
        # count 2 pins everything to 2 engines -- measured 6x slowdown), each
        # an 8 KB-per-partition contiguous read, issued on the two independent
        # HWDGE rings (SP + ACT) to parallelize descriptor generation.
        # E column 2t + r.
        nt = nj // 2
        for t in range(nt):
            ck = chunks.tile([128, 2, H], f32, tag="ck")
            for ds in range(2):
                src = bass.AP(
                    tensor=enc.tensor,
                    offset=enc.offset + (4 * t + 2 * ds) * H,
                    ap=[[s_loc * H, B], [1, 2 * H]])
                eng = nc.sync if ds == 0 else nc.scalar
                eng.dma_start(out=ck[ds * 64:(ds + 1) * 64, :, :], in_=src)
            nc.vector.tensor_mul(ck, ck, u2x)
            for r in range(2):
                # ScalarE: E[:, 2t+r] = sum_h(ck[:,r,:] + c2/H) = enc.u + c2
                nc.scalar.activation(ck[:, r, :], ck[:, r, :], Act.Identity,
                                     bias=c2d, scale=1.0,
                                     accum_out=E[:, 2 * t + r:2 * t + r + 1])

        # ---------- phase 2: softmax over b (per s), emit out ----------
        if nj == 128:
            identNJ = ident128
        else:
            identNJ = singles.tile([nj, nj], f32, tag="identNJ")
            make_identity(nc, identNJ)
        etp = psum.tile([nj, 128], f32, tag="etp")
        nc.tensor.transpose(etp, E, ident128)  # ET[j, p]
        et = small.tile([nj, 128], f32, tag="et")
        nc.vector.tensor_copy(et, etp)

        sums = small.tile([nj, 2], f32, tag="sums")
        for half in range(2):
            sub = et[:, half * 64:(half + 1) * 64]
            negm = small.tile([nj, 1], f32, tag=f"negm{half}")
            nc.vector.tensor_reduce(negm, sub, axis=X, op=Alu.max, negate=True)
            nc.scalar.activation(sub, sub, Act.Exp, bias=negm, scale=1.0,
                                 accum_out=sums[:, half:half + 1])
        rs = small.tile([nj, 2], f32, tag="rs")
        nc.vector.reciprocal(rs, sums)
        for half in range(2):
            sub = et[:, half * 64:(half + 1) * 64]
            nc.vector.tensor_scalar_mul(sub, sub, rs[:, half:half + 1])

        ptp = psum.tile([128, nj], f32, tag="ptp")
        nc.tensor.transpose(ptp, et, identNJ)  # P[p, j]
        pt2 = small.tile([128, nj], f32, tag="pt2")
        nc.vector.tensor_copy(pt2, ptp)

        # interleave: O[b, t, ds, r] = P[ds*64+b, 2t+r]  (s = 4t + 2ds + r)
        O = small.tile([64, nj // 2, 2, 2], f32, tag="O")
        for ds in range(2):
            nc.vector.tensor_copy(
                O[:, :, ds, :],
                pt2[ds * 64:(ds + 1) * 64, 0:nj].rearrange(
                    "b (t r) -> b t r", r=2))
        outv = out.rearrange("b (t ds r) -> b t ds r", ds=2, r=2)
        nc.sync.dma_start(out=outv, in_=O)

    nc.compile()
    return nc


def _get_nc():
    if "nc" not in _CACHE:
        _CACHE["nc"] = build_nc()
    return _CACHE["nc"]


def run_spmd(hidden, encoder_output, W, b, **spmd_kwargs):
    from concourse.bass_utils import run_bass_kernel_spmd

    nc = _get_nc()
    hid2d = np.ascontiguousarray(np.asarray(hidden, dtype=np.float32)[0])
    Wn = np.ascontiguousarray(np.asarray(W, dtype=np.float32))
    bn = np.ascontiguousarray(np.asarray(b, dtype=np.float32).reshape(1, H))
    enc = np.asarray(encoder_output, dtype=np.float32)
    in_maps = []
    for c in range(N_CORES):
        in_maps.append({
            "enc": np.ascontiguousarray(enc[:, c * S_LOC:(c + 1) * S_LOC, :]),
            "hid": hid2d,
            "W": Wn,
            "bias": bn,
        })
    return run_bass_kernel_spmd(nc, in_maps, core_ids=list(range(N_CORES)),
                                **spmd_kwargs)


def kernel(hidden, encoder_output, W, b):
    res = run_spmd(hidden, encoder_output, W, b)
    return np.concatenate([res.results[c]["out"] for c in range(N_CORES)], axis=1)


# revision 15
# speedup vs baseline: 1.1381x; 1.1381x over previous
"""Trainium2 Bass kernel for nn_Attn_69776038691596.

reference computes:
    proj     = einsum('bsh,kh->bsk', enc, W) + bias          # (B,S,H)
    energies = einsum('bh,bsh->bs', hid, proj)               # (B,S)
    out      = softmax(energies, axis=0)                     # over batch

Algebraic rewrite (exact in real arithmetic):
    u[b,:] = hid[b,:] @ W          # (B,H)  -- tiny matmul
    c[b]   = hid[b,:] . bias       # (B,)
    energies[b,s] = enc[b,s,:] . u[b,:] + c[b]

This turns a 275-GFLOP matmul into a 0.27-GFLOP weighted reduction that is
bound by reading encoder_output (512 MB) from HBM once.

Sharding: split the S axis (2048 -> 8 x 256) across the 8 cores. The softmax
runs over the batch axis, which every core holds entirely, so no collectives
are needed.

Per-core layout: partition p = ds*64 + b packs (batch, s-parity); each SBUF
tile (128, H) holds enc[b, 2j+ds, :] rows for one j. A single fused DVE
tensor_tensor_reduce per tile computes E[p, j] = sum_h enc*u2 + c2 (the
per-partition scalar init folds in c). Softmax then runs on the tiny
(128,128) energy tile via PE transpose.
"""
import sys

sys.path.insert(0, "/opt/trn_rl_repo")

import numpy as np

B, S, H = 64, 2048, 1024
N_CORES = 8
S_LOC = S // N_CORES  # 256

_CACHE = {}


def build_nc(s_loc=S_LOC):
    """Build + compile the per-core Bass module. s_loc must be divisible by 4."""
    import concourse.bass as bass
    import concourse.bacc as bacc
    import concourse.tile as tile
    from concourse import mybir
    from concourse.masks import make_identity
    from contextlib import ExitStack

    f32 = mybir.dt.float32
    Alu = mybir.AluOpType
    Act = mybir.ActivationFunctionType
    X = mybir.AxisListType.X

    nj = s_loc // 2          # s-pairs per core (E columns)

    nc = bacc.Bacc("TRN2", target_bir_lowering=False, debug=False,
                   num_devices=N_CORES)
    enc = nc.dram_tensor("enc", [B, s_loc, H], f32, kind="ExternalInput").ap()
    hid = nc.dram_tensor("hid", [B, H], f32, kind="ExternalInput").ap()
    W = nc.dram_tensor("W", [H, H], f32, kind="ExternalInput").ap()
    bias = nc.dram_tensor("bias", [1, H], f32, kind="ExternalInput").ap()
    out = nc.dram_tensor("out", [B, s_loc], f32, kind="ExternalOutput").ap()

    with ExitStack() as ctx:
        tc = ctx.enter_context(tile.TileContext(nc))
        singles = ctx.enter_context(tc.tile_pool(name="singles", bufs=1))
        wpool = ctx.enter_context(tc.tile_pool(name="wpool", bufs=2))
        chunks = ctx.enter_context(tc.tile_pool(name="chunks", bufs=6))
        small = ctx.enter_context(tc.tile_pool(name="small", bufs=1))
        psum = ctx.enter_context(tc.tile_pool(name="psum", bufs=2, space="PSUM"))
        psum1 = ctx.enter_context(tc.tile_pool(name="psum1", bufs=1, space="PSUM"))

        # ---------- phase 0: u2 (128,H), c2 (128,1) ----------
        ident64 = singles.tile([64, 64], f32, tag="ident64")
        make_identity(nc, ident64)
        ident128 = singles.tile([128, 128], f32, tag="ident128")
        make_identity(nc, ident128)

        hid_sb = singles.tile([64, H], f32, tag="hid_sb")
        nc.sync.dma_start(out=hid_sb, in_=hid)

        # bias broadcast to 128 partitions (partition-stride-0 DMA)
        bias_b = singles.tile([128, H], f32, tag="bias_b")
        bias_bc = bass.AP(tensor=bias.tensor, offset=bias.offset,
                          ap=[[0, 128]] + list(bias.ap[1:]))
        nc.sync.dma_start(out=bias_b, in_=bias_bc)

        # hid duplicated on partitions 0-63 / 64-127
        hid2 = singles.tile([128, H], f32, tag="hid2")
        nc.sync.dma_start(out=hid2[0:64, :], in_=hid)
        nc.sync.dma_start(out=hid2[64:128, :], in_=hid)

        # c2[p] = hid2[p,:] . bias ; c2d = c2 / H (folded into the
        # per-element activation bias of the reduction below)
        c2 = singles.tile([128, 1], f32, tag="c2")
        scr = singles.tile([128, H], f32, tag="scr")
        nc.vector.tensor_mul(scr, hid2, bias_b)
        nc.vector.tensor_reduce(c2, scr, axis=X, op=Alu.add)
        c2d = singles.tile([128, 1], f32, tag="c2d")
        nc.vector.tensor_scalar_mul(c2d, c2, 1.0 / H)

        # hidT2[k] = [hid[:,k*128:(k+1)*128].T | same] : (128k, 128p)
        hidT2 = []
        for k in range(8):
            pt = psum.tile([128, 64], f32, tag="pt_hidT")
            nc.tensor.transpose(pt, hid_sb[:, k * 128:(k + 1) * 128], ident64)
            st = singles.tile([128, 128], f32, tag=f"hidT2_{k}")
            nc.vector.tensor_copy(st[:, 0:64], pt)
            nc.vector.tensor_copy(st[:, 64:128], pt)
            hidT2.append(st)

        # u2 = [hid@W ; hid@W] : (128, H) via PE, accumulated over k in PSUM
        u2p = psum1.tile([128, H], f32, tag="u2p")
        for k in range(8):
            wk = wpool.tile([128, H], f32, tag="wk")
            nc.sync.dma_start(out=wk, in_=W[k * 128:(k + 1) * 128, :])
            for nh in range(2):
                nc.tensor.matmul(
                    u2p[:, nh * 512:(nh + 1) * 512],
                    lhsT=hidT2[k],
                    rhs=wk[:, nh * 512:(nh + 1) * 512],
                    start=(k == 0), stop=(k == 7))
        # u2x = u2 duplicated along a 2-wide free dim (matches 2-s-row tiles)
        u2x = singles.tile([128, 2, H], f32, tag="u2x")
        nc.vector.tensor_copy(u2x[:, 0, :], u2p)
        nc.vector.tensor_copy(u2x[:, 1, :], u2p)

        # ---------- phase 1: energies E[p, j] ----------
        E = singles.tile([128, nj], f32, tag="E")
        # Tile t holds s = 4t + 2*ds + r : partition p = ds*64 + b, free (r, h).
        # Per tile, two DMAs (one per ds half / partition half) so descriptors
        # fan out across the SDMA engines serving that half (a leading dim of
        # count 2 pins everything to 2 engines -- measured 6x slowdown), each
        # an 8 KB-per-partition contiguous read, issued on the two independent
        # HWDGE rings (SP + ACT) to parallelize descriptor generation.
        # E column 2t + r.
        nt = nj // 2
        for t in range(nt):
            ck = chunks.tile([128, 2, H], f32, tag="ck")
            for ds in range(2):
                src = bass.AP(
                    tensor=enc.tensor,
                    offset=enc.offset + (4 * t + 2 * ds) * H,
                    ap=[[s_loc * H, B], [1, 2 * H]])
                nc.sync.dma_start(out=ck[ds * 64:(ds + 1) * 64, :, :], in_=src)
            nc.vector.tensor_mul(ck, ck, u2x)
            for r in range(2):
                # ScalarE: E[:, 2t+r] = sum_h(ck[:,r,:] + c2/H) = enc.u + c2
                nc.scalar.activation(ck[:, r, :], ck[:, r, :], Act.Identity,
                                     bias=c2d, scale=1.0,
                                     accum_out=E[:, 2 * t + r:2 * t + r + 1])

        # ---------- phase 2: softmax over b (per s), emit out ----------
        if nj == 128:
            identNJ = ident128
        else:
            identNJ = singles.tile([nj, nj], f32, tag="identNJ")
            make_identity(nc, identNJ)
        etp = psum.tile([nj, 128], f32, tag="etp")
        nc.tensor.transpose(etp, E, ident128)  # ET[j, p]
        et = small.tile([nj, 128], f32, tag="et")
        nc.vector.tensor_copy(et, etp)

        sums = small.tile([nj, 2], f32, tag="sums")
        for half in range(2):
            sub = et[:, half * 64:(half + 1) * 64]
            negm = small.tile([nj, 1], f32, tag=f"negm{half}")
            nc.vector.tensor_reduce(negm, sub, axis=X, op=Alu.max, negate=True)
            nc.scalar.activation(sub, sub, Act.Exp, bias=negm, scale=1.0,
                                 accum_out=sums[:, half:half + 1])
        rs = small.tile([nj, 2], f32, tag="rs")
        nc.vector.reciprocal(rs, sums)
        for half in range(2):
            sub = et[:, half * 64:(half + 1) * 64]
            nc.vector.tensor_scalar_mul(sub, sub, rs[:, half:half + 1])

        ptp = psum.tile([128, nj], f32, tag="ptp")
        nc.tensor.transpose(ptp, et, identNJ)  # P[p, j]
        pt2 = small.tile([128, nj], f32, tag="pt2")
        nc.vector.tensor_copy(pt2, ptp)

        # interleave: O[b, t, ds, r] = P[ds*64+b, 2t+r]  (s = 4t + 2ds + r)
        O = small.tile([64, nj // 2, 2, 2], f32, tag="O")
        for ds in range(2):
            nc.vector.tensor_copy(
                O[:, :, ds, :],
                pt2[ds * 64:(ds + 1) * 64, 0:nj].rearrange(
                    "b (t r) -> b t r", r=2))
        outv = out.rearrange("b (t ds r) -> b t ds r", ds=2, r=2)
        nc.sync.dma_start(out=outv, in_=O)

    nc.compile()
    return nc


def _get_nc():
    if "nc" not in _CACHE:
        _CACHE["nc"] = build_nc()
    return _CACHE["nc"]


def run_spmd(hidden, encoder_output, W, b, **spmd_kwargs):
    from concourse.bass_utils import run_bass_kernel_spmd

    nc = _get_nc()
    hid2d = np.ascontiguousarray(np.asarray(hidden, dtype=np.float32)[0])
    Wn = np.ascontiguousarray(np.asarray(W, dtype=np.float32))
    bn = np.ascontiguousarray(np.asarray(b, dtype=np.float32).reshape(1, H))
    enc = np.asarray(encoder_output, dtype=np.float32)
    in_maps = []
    for c in range(N_CORES):
        in_maps.append({
            "enc": np.ascontiguousarray(enc[:, c * S_LOC:(c + 1) * S_LOC, :]),
            "hid": hid2d,
            "W": Wn,
            "bias": bn,
        })
    return run_bass_kernel_spmd(nc, in_maps, core_ids=list(range(N_CORES)),
                                **spmd_kwargs)


def kernel(hidden, encoder_output, W, b):
    res = run_spmd(hidden, encoder_output, W, b)
    return np.concatenate([res.results[c]["out"] for c in range(N_CORES)], axis=1)


# revision 19
# speedup vs baseline: 1.4605x; 1.2834x over previous
"""Trainium2 Bass kernel for nn_Attn_69776038691596.

reference computes:
    proj     = einsum('bsh,kh->bsk', enc, W) + bias          # (B,S,H)
    energies = einsum('bh,bsh->bs', hid, proj)               # (B,S)
    out      = softmax(energies, axis=0)                     # over batch

Algebraic rewrite (exact in real arithmetic):
    u[b,:] = hid[b,:] @ W          # (B,H)  -- tiny matmul
    c[b]   = hid[b,:] . bias       # (B,)
    energies[b,s] = enc[b,s,:] . u[b,:] + c[b]

This turns a 275-GFLOP matmul into a 0.27-GFLOP weighted reduction that is
bound by reading encoder_output (512 MB) from HBM once.

Sharding: split the S axis (2048 -> 8 x 256) across the 8 cores. The softmax
runs over the batch axis, which every core holds entirely, so no collectives
are needed.

Per-core schedule (all contiguous HBM streams; gather layouts measured ~2x
slower on the HBM read side):
  - phase 0 (tiny): u = hid @ W on PE; c_row = bias . hid^T on PE;
    cb2[s,b] = c[b]/H broadcast via K=1 ones-matmul.
  - main loop over b: stream enc[b] as two contiguous (128s, H) tiles;
    broadcast u[b] over partitions via K=1 ones-matmul into PSUM;
    DVE multiplies, ScalarE activation-accumulates into E_half[s, b]
    (the c[b]/H activation bias folds in the energy offset).
  - softmax over the free (b) axis of E_half directly; PE-transpose the
    (128s, 64b) result to (64b, 128s) and stream out rows.
"""
import sys

sys.path.insert(0, "/opt/trn_rl_repo")

import numpy as np

B, S, H = 64, 2048, 1024
N_CORES = 8
S_LOC = S // N_CORES  # 256

_CACHE = {}


def build_nc(s_loc=S_LOC):
    """Build + compile the per-core Bass module. s_loc must be divisible by 128."""
    import concourse.bass as bass
    import concourse.bacc as bacc
    import concourse.tile as tile
    from concourse import mybir
    from concourse.masks import make_identity
    from contextlib import ExitStack

    f32 = mybir.dt.float32
    Alu = mybir.AluOpType
    Act = mybir.ActivationFunctionType
    X = mybir.AxisListType.X

    nhalf = s_loc // 128     # (128,H) tiles per batch row

    nc = bacc.Bacc("TRN2", target_bir_lowering=False, debug=False,
                   num_devices=N_CORES)
    enc = nc.dram_tensor("enc", [B, s_loc, H], f32, kind="ExternalInput").ap()
    hid = nc.dram_tensor("hid", [B, H], f32, kind="ExternalInput").ap()
    W = nc.dram_tensor("W", [H, H], f32, kind="ExternalInput").ap()
    bias = nc.dram_tensor("bias", [1, H], f32, kind="ExternalInput").ap()
    out = nc.dram_tensor("out", [B, s_loc], f32, kind="ExternalOutput").ap()

    with ExitStack() as ctx:
        tc = ctx.enter_context(tile.TileContext(nc))
        singles = ctx.enter_context(tc.tile_pool(name="singles", bufs=1))
        wpool = ctx.enter_context(tc.tile_pool(name="wpool", bufs=2))
        chunks = ctx.enter_context(tc.tile_pool(name="chunks", bufs=8))
        small = ctx.enter_context(tc.tile_pool(name="small", bufs=1))
        psum = ctx.enter_context(tc.tile_pool(name="psum", bufs=2, space="PSUM"))
        psum1 = ctx.enter_context(tc.tile_pool(name="psum1", bufs=1, space="PSUM"))
        psumB = ctx.enter_context(tc.tile_pool(name="psumB", bufs=2, space="PSUM"))

        # ---------- phase 0 ----------
        ident64 = singles.tile([64, 64], f32, tag="ident64")
        make_identity(nc, ident64)
        ident128 = singles.tile([128, 128], f32, tag="ident128")
        make_identity(nc, ident128)
        ones1 = singles.tile([1, 128], f32, tag="ones1")
        nc.vector.memset(ones1, 1.0)

        hid_sb = singles.tile([64, H], f32, tag="hid_sb")
        nc.sync.dma_start(out=hid_sb, in_=hid)

        # hidT[k] : (128k, 64b) via PE transpose
        hidT = []
        for k in range(8):
            pt = psum.tile([128, 64], f32, tag="pp")
            nc.tensor.transpose(pt, hid_sb[:, k * 128:(k + 1) * 128], ident64)
            st = singles.tile([128, 64], f32, tag=f"hidT_{k}")
            nc.vector.tensor_copy(st, pt)
            hidT.append(st)

        # bias as (128,1) per k-chunk; c_row = sum_k bias_k^T @ hidT_k : (1, 64)
        bias_sb = singles.tile([128, 8], f32, tag="bias_sb")
        nc.sync.dma_start(
            out=bias_sb,
            in_=bass.AP(tensor=bias.tensor, offset=bias.offset,
                        ap=[[1, 128], [128, 8]]))
        c_psum = psum.tile([1, 64], f32, tag="pp")
        for k in range(8):
            nc.tensor.matmul(c_psum, lhsT=bias_sb[:, k:k + 1], rhs=hidT[k],
                             start=(k == 0), stop=(k == 7))
        c_row = singles.tile([1, 64], f32, tag="c_row")
        nc.vector.tensor_scalar_mul(c_row, c_psum, 1.0 / H)

        # cb2[s, b] = c[b]/H on all 128 partitions (K=1 ones-matmul)
        cb_psum = psum.tile([128, 64], f32, tag="pp")
        nc.tensor.matmul(cb_psum, lhsT=ones1, rhs=c_row, start=True, stop=True)
        cb2 = singles.tile([128, 64], f32, tag="cb2")
        nc.vector.tensor_copy(cb2, cb_psum)

        # u = hid @ W : (64, H) via PE, accumulated over k in PSUM
        u_psum = psum1.tile([64, H], f32, tag="u_psum")
        for k in range(8):
            wk = wpool.tile([128, H], f32, tag="wk")
            nc.sync.dma_start(out=wk, in_=W[k * 128:(k + 1) * 128, :])
            for nh in range(2):
                nc.tensor.matmul(
                    u_psum[:, nh * 512:(nh + 1) * 512],
                    lhsT=hidT[k][:, 0:64],
                    rhs=wk[:, nh * 512:(nh + 1) * 512],
                    start=(k == 0), stop=(k == 7))
        u_sb = singles.tile([64, H], f32, tag="u_sb")
        nc.vector.tensor_copy(u_sb, u_psum)

        # ---------- phase 1: energies E_half[s, b] ----------
        Eh = [singles.tile([128, B], f32, tag=f"E{i}", name=f"E{i}")
              for i in range(nhalf)]
        for b in range(B):
            # ub[s, h] = u[b, h] on all partitions: selector matmul
            # lhsT[k, m] = ident64[k, b] (free-dim stride-0 broadcast), so
            # out[m, h] = sum_k ident64[k, b] * u[k, h] = u[b, h].
            selcol = ident64[0:64, b:b + 1]
            sel_b = bass.AP(tensor=selcol.tensor, offset=selcol.offset,
                            ap=[list(selcol.ap[0]), [0, 128]])
            ub = psumB.tile([128, H], f32, tag="ub")
            for nh in range(2):
                nc.tensor.matmul(ub[:, nh * 512:(nh + 1) * 512],
                                 lhsT=sel_b,
                                 rhs=u_sb[0:64, nh * 512:(nh + 1) * 512],
                                 start=True, stop=True)
            for i in range(nhalf):
                ck = chunks.tile([128, H], f32, tag="ck")
                nc.sync.dma_start(out=ck, in_=enc[b, i * 128:(i + 1) * 128, :])
                nc.vector.tensor_mul(ck, ck, ub)
                # ScalarE: E[s, b] = sum_h(ck + c[b]/H) = enc[b,s,:].u + c[b]
                nc.scalar.activation(ck, ck, Act.Identity,
                                     bias=cb2[:, b:b + 1], scale=1.0,
                                     accum_out=Eh[i][:, b:b + 1])

        # ---------- phase 2: softmax over b (free axis), emit out ----------
        for i in range(nhalf):
            e = Eh[i]
            negm = small.tile([128, 1], f32, tag=f"negm{i}")
            nc.vector.tensor_reduce(negm, e, axis=X, op=Alu.max, negate=True)
            ssum = small.tile([128, 1], f32, tag=f"ssum{i}")
            nc.scalar.activation(e, e, Act.Exp, bias=negm, scale=1.0,
                                 accum_out=ssum)
            rs = small.tile([128, 1], f32, tag=f"rs{i}")
            nc.vector.reciprocal(rs, ssum)
            nc.vector.tensor_scalar_mul(e, e, rs)
            # transpose (128s, 64b) -> (64b, 128s), stream out
            op = psum.tile([64, 128], f32, tag="pp")
            nc.tensor.transpose(op, e, ident128)
            ot = small.tile([64, 128], f32, tag=f"ot{i}")
            nc.vector.tensor_copy(ot, op)
            nc.sync.dma_start(out=out[:, i * 128:(i + 1) * 128], in_=ot)

    nc.compile()
    return nc


def _get_nc():
    if "nc" not in _CACHE:
        _CACHE["nc"] = build_nc()
    return _CACHE["nc"]


def run_spmd(hidden, encoder_output, W, b, **spmd_kwargs):
    from concourse.bass_utils import run_bass_kernel_spmd

    nc = _get_nc()
    hid2d = np.ascontiguousarray(np.asarray(hidden, dtype=np.float32)[0])
    Wn = np.ascontiguousarray(np.asarray(W, dtype=np.float32))
    bn = np.ascontiguousarray(np.asarray(b, dtype=np.float32).reshape(1, H))
    enc = np.asarray(encoder_output, dtype=np.float32)
    in_maps = []
    for c in range(N_CORES):
        in_maps.append({
            "enc": np.ascontiguousarray(enc[:, c * S_LOC:(c + 1) * S_LOC, :]),
            "hid": hid2d,
            "W": Wn,
            "bias": bn,
        })
    return run_bass_kernel_spmd(nc, in_maps, core_ids=list(range(N_CORES)),
                                **spmd_kwargs)


def kernel(hidden, encoder_output, W, b):
    res = run_spmd(hidden, encoder_output, W, b)
    return np.concatenate([res.results[c]["out"] for c in range(N_CORES)], axis=1)
